# revision 1
# baseline (speedup 1.0000x reference)
"""Trainium2 Bass kernel for nn_NERModel loss (CE + quadruplet + context MSE).

Strategy (8 NeuronCores, data-parallel over batch):
  - Each core processes 8 batches = 8192 tokens of embeddings [8192, 384] f32.
  - Tokens are tiled 128/tile at stride 127 (65 tiles) so every adjacent-token
    pair falls inside some tile; host-built per-(tile,slot) weights de-dup
    overlapping tokens/pairs exactly once.
  - CE: PE transposes emb chunks (PSUM), ScE/VE copy to SBUF, then
    logitsT[17,512] = W.T-chunk (lhsT) @ embT (rhs) accumulated over 3 K-chunks.
    exp on ScE with per-partition bias=b (free bias add), per-token sel via a
    fused tensor_tensor_reduce against a host-built ce_w-scaled one-hot, and
    sumexp column sums via an accumulating row-placement matmul into one
    persistent PSUM bank. One ln at the end.
  - CTX: one matmul per tile with constant (S - I) weights produces adjacent
    diffs straight into PSUM; fused square+pair-weight+row-reduce split across
    ScE (activation Square, scale=w, accum_out) and VE (tensor_tensor_reduce).
  - Device returns two partial sums per core; host does the tiny quadruplet
    term (index scans over labels + 49 gathered rows) and final combination.
"""

import sys

for _p in ("/opt/trn_rl_repo", "/root/.axon_site/_ro/trn_rl_repo"):
    if _p not in sys.path:
        sys.path.append(_p)

import numpy as np
from contextlib import ExitStack

import concourse.bass as bass
import concourse.bacc as bacc
import concourse.mybir as mybir
from concourse import tile
from concourse.ap import AP

NUM_LABELS = 17
MARGIN = 1.0
IGNORE = -100

B, S, H, L = 64, 1024, 384, NUM_LABELS
NCORES = 8
BP = B // NCORES            # batches per core
NTOK = BP * S               # tokens per core (8192)
STRIDE = 127                # token stride between tiles (1-token overlap)
NT = 65                     # tiles per core
NG = (NT + 3) // 4          # compute groups of 4 tiles -> 17
GDMA = 8                    # tiles per DMA transfer
NDMA = (NT + GDMA - 1) // GDMA  # 9
F32 = mybir.dt.float32


def _tile_start(t: int) -> int:
    # last tile is clamped so it stays in-bounds; duplicated tokens/pairs are
    # zero-weighted on the host side
    return NTOK - 128 if t == NT - 1 else STRIDE * t


def _build_nc() -> bass.Bass:
    import os

    skip_ctx = bool(os.environ.get("NER_SKIP_CTX"))
    skip_ce = bool(os.environ.get("NER_SKIP_CE"))
    no_gpsimd = bool(os.environ.get("NER_NO_GPSIMD_MEMSET"))
    # Bacc (not plain Bass): its compile() legalizes sync waits (>=2 waits per
    # instruction are split / moved to LDWEIGHTS), which walrus requires.
    nc = bacc.Bacc("TRN2", debug=False)

    emb = nc.declare_dram_parameter("emb", [NTOK, H], F32, isOutput=False)
    woh = nc.declare_dram_parameter("woh", [L, NG * 512], F32, isOutput=False)
    cewg = nc.declare_dram_parameter("cewg", [NG, 512], F32, isOutput=False)
    pairw = nc.declare_dram_parameter("pairw", [128, NT], F32, isOutput=False)
    wt = nc.declare_dram_parameter("wt", [128, 3 * L], F32, isOutput=False)
    bcol = nc.declare_dram_parameter("bcol", [L, 1], F32, isOutput=False)
    selg = nc.declare_dram_parameter("selg", [L, NG * L], F32, isOutput=False)
    dfw = nc.declare_dram_parameter("dfw", [128, 128], F32, isOutput=False)
    idn = nc.declare_dram_parameter("idn", [128, 128], F32, isOutput=False)
    ones = nc.declare_dram_parameter("ones", [128, 1], F32, isOutput=False)
    outv = nc.declare_dram_parameter("outv", [1, 8], F32, isOutput=True)

    AF = mybir.ActivationFunctionType
    AX = mybir.AxisListType
    OP = mybir.AluOpType

    with tile.TileContext(nc) as tc, ExitStack() as ctx:
        consts = ctx.enter_context(tc.tile_pool(name="consts", bufs=1))
        nat_pool = ctx.enter_context(tc.tile_pool(name="nat", bufs=3))
        embt_pool = ctx.enter_context(tc.tile_pool(name="embt", bufs=2))
        expt_pool = ctx.enter_context(tc.tile_pool(name="expt", bufs=2))
        junk_pool = ctx.enter_context(tc.tile_pool(name="junk", bufs=2))
        acc_pool = ctx.enter_context(tc.tile_pool(name="acc", bufs=1))
        ps_t = ctx.enter_context(tc.tile_pool(name="ps_t", bufs=1, space="PSUM"))
        ps_l = ctx.enter_context(tc.tile_pool(name="ps_l", bufs=1, space="PSUM"))
        ps_d = ctx.enter_context(tc.tile_pool(name="ps_d", bufs=1, space="PSUM"))
        ps_s = ctx.enter_context(tc.tile_pool(name="ps_s", bufs=1, space="PSUM"))

        def cload(handle, shape):
            t = consts.tile(list(shape), F32, tag=handle.name + "_c")
            nc.sync.dma_start(out=t[:], in_=handle.ap())
            return t

        woh_t = cload(woh, (L, NG * 512))
        cewg_t = cload(cewg, (NG, 512))
        pairw_t = cload(pairw, (128, NT))
        wt_t = cload(wt, (128, 3 * L))
        bcol_t = cload(bcol, (L, 1))
        selg_t = cload(selg, (L, NG * L))
        dfw_t = cload(dfw, (128, 128))
        idn_t = cload(idn, (128, 128))
        ones_t = cload(ones, (128, 1))

        # persistent accumulators
        sumexp_ps = ps_s.tile([L, 512], F32)          # [group, group-token]
        ctxbuf = acc_pool.tile([128, NT], F32)        # per-tile weighted ||diff||^2
        selbuf = acc_pool.tile([L, NG], F32)          # per-group sum cew*logit
        nc.vector.memset(selbuf[:], 0.0)

        nat_tiles = {}

        simple_dma = bool(os.environ.get("NER_SIMPLE_DMA"))
        skip_emb_dma = bool(os.environ.get("NER_SKIP_EMB_DMA"))

        def do_dma(d: int):
            ntl = min(GDMA, NT - d * GDMA)
            nat = nat_pool.tile([128, GDMA * H], F32, tag="natbuf")
            if skip_emb_dma:
                nat_tiles[d] = nat
                return
            if simple_dma:
                for j in range(ntl):
                    src = AP(
                        tensor=emb,
                        offset=_tile_start(d * GDMA + j) * H,
                        ap=[[H, 128], [1, H]],
                    )
                    nc.sync.dma_start(out=nat[:, j * H : (j + 1) * H], in_=src)
            elif ntl == GDMA:
                src = AP(
                    tensor=emb,
                    offset=_tile_start(d * GDMA) * H,
                    ap=[[H, 128], [STRIDE * H, GDMA], [1, H]],
                )
                nc.sync.dma_start(out=nat[:, :].rearrange("p (g h) -> p g h", h=H), in_=src)
            else:
                src = AP(
                    tensor=emb,
                    offset=_tile_start(d * GDMA) * H,
                    ap=[[H, 128], [1, H]],
                )
                nc.sync.dma_start(out=nat[:, 0:H], in_=src)
            nat_tiles[d] = nat

        def nat_slice(t: int, c0: int, c1: int):
            nat = nat_tiles[t // GDMA]
            base = (t % GDMA) * H
            return nat[:, base + c0 : base + c1]

        def _ctx_only(tiles):
            for half in range(2):
                tiles_h = tiles[2 * half : 2 * half + 2]
                if not tiles_h:
                    break
                df_ps = ps_d.tile([128, 2, 512], F32, tag="df_ps")
                for jj, t in enumerate(tiles_h):
                    nc.tensor.matmul(
                        df_ps[:, jj, 0:H], dfw_t[:], nat_slice(t, 0, H),
                        start=True, stop=True,
                    )
                _sqw(tiles_h, df_ps)

        def do_group(g: int):
            tiles = list(range(4 * g, min(4 * g + 4, NT)))
            last = len(tiles) < 4

            # ---- transposes: embT[h, tok] chunks ----
            if skip_ce:
                _ctx_only(tiles)
                return
            embT_ps = ps_t.tile([128, 3 * 512], F32, tag="embT_ps")
            for j, t in enumerate(tiles):
                for c in range(3):
                    # out = nat_chunk.T via a normal matmul against identity
                    # (transpose-mode LW has too few sync-wait slots in codegen)
                    nc.tensor.matmul(
                        embT_ps[:, c * 512 + j * 128 : c * 512 + (j + 1) * 128],
                        nat_slice(t, c * 128, (c + 1) * 128),
                        idn_t[:],
                        start=True,
                        stop=True,
                    )
            embT = embt_pool.tile([128, 3 * 512], F32, tag="embT")
            if last:
                # only j=0 columns are real; zero the rest so downstream
                # full-width ops read finite garbage
                (nc.vector if no_gpsimd else nc.gpsimd).memset(embT[:], 0.0)
                ev = embT[:, :].rearrange("p (c k) -> p c k", k=512)
                pv = embT_ps[:, :].rearrange("p (c k) -> p c k", k=512)
                nc.vector.tensor_copy(ev[:, :, 0:128], pv[:, :, 0:128])
            else:
                nc.vector.tensor_copy(embT[:], embT_ps[:])

            # ---- logitsT [17, 512] ----
            lg_ps = ps_l.tile([L, 512], F32, tag="lg_ps")
            for c in range(3):
                nc.tensor.matmul(
                    lg_ps[:],
                    wt_t[:, c * L : (c + 1) * L],
                    embT[:, c * 512 : (c + 1) * 512],
                    start=(c == 0),
                    stop=(c == 2),
                )

            # ---- exp(logit + b) ----
            expT = expt_pool.tile([L, 512], F32, tag="expT")
            nc.scalar.activation(expT[:], lg_ps[:], AF.Exp, bias=bcol_t[:, 0:1], scale=1.0)

            # ---- sel accumulation: selacc += sum(logit * woh) ----
            junk17 = junk_pool.tile([L, 512], F32, tag="junk17")
            nc.vector.tensor_mul(junk17[:], lg_ps[:], woh_t[:, g * 512 : (g + 1) * 512])
            junk17c = junk_pool.tile([L, 512], F32, tag="junk17b")
            nc.vector.tensor_scalar(
                out=junk17c[:], in0=junk17[:], scalar1=1.0, scalar2=None,
                op0=OP.mult, op1=OP.add, accum_out=selbuf[:, g : g + 1],
            )

            # ---- sumexp row-placement matmul ----
            nc.tensor.matmul(
                sumexp_ps[:],
                selg_t[:, g * L : (g + 1) * L],
                expT[:],
                start=(g == 0),
                stop=(g == NG - 1),
            )

            # ---- ctx: diff = emb[t+1]-emb[t] via (S-I) matmul, then w*||diff||^2 ----
            # each matmul output must live inside one 512-col PSUM bank, so
            # pad each tile's diff region to 512 and process 2 tiles per alloc
            if skip_ctx:
                return
            for half in range(2):
                tiles_h = tiles[2 * half : 2 * half + 2]
                if not tiles_h:
                    break
                df_ps = ps_d.tile([128, 2, 512], F32, tag="df_ps")
                for jj, t in enumerate(tiles_h):
                    nc.tensor.matmul(
                        df_ps[:, jj, 0:H],
                        dfw_t[:],
                        nat_slice(t, 0, H),
                        start=True,
                        stop=True,
                    )
                _sqw(tiles_h, df_ps)

        def _sqw(tiles_h, df_ps):
            for jj, t in enumerate(tiles_h):
                dsl = df_ps[:, jj, 0:H]
                if False:
                    pass
                else:
                    jk = junk_pool.tile([128, H], F32, tag="junkS")
                    nc.scalar.activation(
                        jk[:],
                        dsl,
                        AF.Square,
                        bias=0.0,
                        scale=pairw_t[:, t : t + 1],
                        accum_out=ctxbuf[:, t : t + 1],
                    )

        g_done = 0
        for d in range(NDMA):
            do_dma(d)
            # run all compute groups fully covered by the DMAs issued so far
            tiles_ready = min((d + 1) * GDMA, NT)
            while g_done < NG and min(4 * g_done + 4, NT) <= tiles_ready:
                do_group(g_done)
                g_done += 1
        assert g_done == NG

        # ---- final reduction ----
        skip_final = bool(os.environ.get("NER_SKIP_FINAL"))
        if skip_final:
            outs0 = acc_pool.tile([1, 8], F32)
            nc.vector.memset(outs0[:], 0.0)
            nc.sync.dma_start(out=outv.ap(), in_=outs0[:])
        if skip_ce:
            nc.vector.memset(sumexp_ps[:], 1.0)
        if skip_ctx:
            nc.vector.memset(ctxbuf[:], 0.0)
        if not skip_final:
            lnsum = expt_pool.tile([L, 512], F32, tag="lnsum")
            nc.scalar.activation(lnsum[:], sumexp_ps[:], AF.Ln)
            accA = acc_pool.tile([L, 1], F32)
            junk17b = junk_pool.tile([L, 512], F32, tag="junk17")
            nc.vector.tensor_mul(junk17b[:], lnsum[:], cewg_t[:])
            junk17d = junk_pool.tile([L, 512], F32, tag="junk17b")
            nc.vector.tensor_scalar(
                out=junk17d[:], in0=junk17b[:], scalar1=1.0, scalar2=None,
                op0=OP.mult, op1=OP.add, accum_out=accA[:, 0:1],
            )
            selacc = acc_pool.tile([L, 1], F32)
            junkS = junk_pool.tile([L, NG], F32, tag="junkS17")
            nc.vector.tensor_scalar(
                out=junkS[:], in0=selbuf[:], scalar1=1.0, scalar2=None,
                op0=OP.mult, op1=OP.add, accum_out=selacc[:, 0:1],
            )
            cev = acc_pool.tile([L, 1], F32)
            nc.vector.tensor_sub(cev[:], accA[:], selacc[:])
            fin1 = ps_l.tile([1, 1], F32, tag="lg_ps")
            nc.tensor.matmul(fin1[:], cev[:], ones_t[0:L, :], start=True, stop=True)

            ctxsum = acc_pool.tile([128, 1], F32)
            nc.vector.tensor_reduce(ctxsum[:], ctxbuf[:], axis=AX.X, op=OP.add)
            fin2 = ps_l.tile([1, 1], F32, tag="lg_ps")
            nc.tensor.matmul(fin2[:], ctxsum[:], ones_t[:], start=True, stop=True)

            outs = acc_pool.tile([1, 8], F32)
            nc.vector.memset(outs[:], 0.0)
            nc.scalar.copy(outs[0:1, 0:1], fin1[:])
            nc.scalar.copy(outs[0:1, 1:2], fin2[:])
            nc.sync.dma_start(out=outv.ap(), in_=outs[:])

    nc.compile()
    return nc


# ---------------------------------------------------------------------------
# host-side preparation


def _host_grids(labf: np.ndarray, mskf: np.ndarray):
    """Per-core grids. labf/mskf: [NTOK] int64/int32.

    Returns (cew_grid [NT,128], pairw_grid [NT,128], woh [L, NG*512],
             cewg [NG, 512])."""
    valid = labf != IGNORE
    pair_ok = np.zeros(NTOK, dtype=bool)
    lf = labf.astype(np.int64)
    # pair (k, k+1) within a batch row of length S
    k = np.arange(NTOK - 1)
    in_batch = (k % S) != (S - 1)
    pair_ok[:-1] = in_batch & (lf[:-1] != IGNORE) & (lf[:-1] == lf[1:]) & (lf[:-1] > 0)

    cew_grid = np.zeros((NT, 128), np.float32)
    pairw_grid = np.zeros((NT, 128), np.float32)
    seen_tok = np.zeros(NTOK, dtype=bool)
    seen_pair = np.zeros(NTOK, dtype=bool)
    tokmap = np.zeros((NT, 128), np.int64)
    for t in range(NT):
        s0 = _tile_start(t)
        toks = np.arange(s0, s0 + 128)
        tokmap[t] = toks
        fresh = ~seen_tok[toks]
        cew_grid[t] = (valid[toks] & fresh).astype(np.float32)
        seen_tok[toks] = True
        pfresh = ~seen_pair[toks]
        pw = pair_ok[toks] & pfresh
        pw[127] = False  # col 127 diff is garbage by construction
        pairw_grid[t] = pw.astype(np.float32)
        seen_pair[toks[:127]] = True

    woh = np.zeros((L, NG * 512), np.float32)
    cewg = np.zeros((NG, 512), np.float32)
    for g in range(NG):
        for j in range(min(4, NT - 4 * g)):
            t = 4 * g + j
            toks = tokmap[t]
            cols = g * 512 + j * 128 + np.arange(128)
            cewg[g, j * 128 : (j + 1) * 128] = cew_grid[t]
            lab_c = np.where(valid[toks], lf[toks], 0)
            woh[lab_c, cols] = cew_grid[t]
    return cew_grid, pairw_grid, woh, cewg


def _quad_host(fe: np.ndarray, fl: np.ndarray, fm: np.ndarray) -> np.float32:
    """Mirror of the reference quadruplet loss in numpy float32."""
    N = fe.shape[0]
    idx = np.arange(N, dtype=np.int64)
    BIG = N
    fm_b = fm > 0
    is_ent = fm_b & (fl > 0)
    non_ent = fm_b & (fl == 0)
    d_i = np.min(np.where(non_ent, idx, BIG))
    has_non = bool(non_ent.any())

    a_i = np.zeros(L - 1, np.int64)
    p_i = np.zeros(L - 1, np.int64)
    n_i = np.zeros(L - 1, np.int64)
    ok = np.zeros(L - 1, bool)
    for i, t in enumerate(range(1, L)):
        m = is_ent & (fl == t)
        order = np.sort(np.where(m, idx, BIG))
        a_i[i], p_i[i] = order[0], order[1]
        cnt = int(m.sum())
        other = is_ent & (fl != t)
        n_i[i] = np.min(np.where(other, idx, BIG))
        ok[i] = (cnt >= 2) and bool(other.any()) and has_non

    clip = lambda v: np.clip(v, 0, N - 1)
    A = fe[clip(a_i)]
    P = fe[clip(p_i)]
    Ng = fe[clip(n_i)]
    D = fe[clip(np.array([d_i]))]
    eps = np.float32(1e-6)

    def dist(x, y):
        d = (x - y + eps).astype(np.float32)
        return np.sqrt(np.sum(d * d, axis=-1, dtype=np.float32)).astype(np.float32)

    pd, nd, dd = dist(A, P), dist(A, Ng), dist(A, D)
    ql = np.maximum(pd - nd + np.float32(MARGIN), 0) + np.maximum(
        pd - dd + np.float32(2.0 * MARGIN), 0
    )
    qcnt = int(ok.sum())
    quad = float(np.sum(np.where(ok, ql, 0.0), dtype=np.float64)) / max(qcnt, 1)
    return np.float32(quad if qcnt > 0 else 0.0)


_NC_CACHE = {}


def _get_nc():
    if "nc" not in _NC_CACHE:
        _NC_CACHE["nc"] = _build_nc()
    return _NC_CACHE["nc"]


def _device_consts():
    if "consts" in _NC_CACHE:
        return _NC_CACHE["consts"]
    dfw = np.zeros((128, 128), np.float32)
    for t in range(127):
        dfw[t + 1, t] = 1.0
    dfw[np.arange(128), np.arange(128)] -= 1.0
    idn = np.eye(128, dtype=np.float32)
    ones = np.ones((128, 1), np.float32)
    selg = np.zeros((L, NG * L), np.float32)
    for g in range(NG):
        selg[:, g * L + g] = 1.0
    _NC_CACHE["consts"] = (dfw, idn, ones, selg)
    return _NC_CACHE["consts"]


def kernel(embeddings, classifier_w, classifier_b, labels, attention_mask):
    from concourse.bass_utils import run_bass_kernel_spmd

    emb = np.ascontiguousarray(np.asarray(embeddings, dtype=np.float32))
    W = np.asarray(classifier_w, dtype=np.float32)
    b = np.asarray(classifier_b, dtype=np.float32)
    lab = np.asarray(labels)
    msk = np.asarray(attention_mask)

    lab_f = lab.reshape(-1).astype(np.int64)
    msk_f = msk.reshape(-1).astype(np.int64)
    N = B * S

    wt = np.zeros((128, 3 * L), np.float32)
    for c in range(3):
        wt[:, c * L : (c + 1) * L] = W[:, c * 128 : (c + 1) * 128].T
    bcol = b.reshape(L, 1).astype(np.float32)
    dfw, idn, ones, selg = _device_consts()

    in_maps = []
    cew_grids = []
    for cidx in range(NCORES):
        sl = slice(cidx * NTOK, (cidx + 1) * NTOK)
        labc = lab_f[sl]
        cewg_grid, pairw_grid, woh, cewg = _host_grids(labc, msk_f[sl])
        cew_grids.append(cewg_grid)
        in_maps.append(
            {
                "emb": emb.reshape(N, H)[sl],
                "woh": woh,
                "cewg": cewg,
                "pairw": np.ascontiguousarray(pairw_grid.T),
                "wt": wt,
                "bcol": bcol,
                "selg": selg,
                "dfw": dfw,
                "idn": idn,
                "ones": ones,
            }
        )

    nc = _get_nc()
    res = run_bass_kernel_spmd(nc, in_maps, list(range(NCORES)))

    ce_sum = 0.0
    ctx_sum = 0.0
    for cidx in range(NCORES):
        out = res.results[cidx]["outv"]
        ce_sum += float(out[0, 0])
        ctx_sum += float(out[0, 1])

    valid = lab_f != IGNORE
    ce_cnt = int(valid.sum())
    # device sel used logits without bias; correct with sum(cew * b[label])
    lab_safe = np.where(valid, lab_f, 0)
    ce_sum -= float(np.sum(np.where(valid, b[lab_safe], 0.0), dtype=np.float64))
    ce = ce_sum / max(ce_cnt, 1)

    pair_ok = np.zeros(N, dtype=bool)
    k = np.arange(N - 1)
    in_batch = (k % S) != (S - 1)
    pair_ok[:-1] = (
        in_batch & (lab_f[:-1] != IGNORE) & (lab_f[:-1] == lab_f[1:]) & (lab_f[:-1] > 0)
    )
    pc = int(pair_ok.sum())
    ctx = (ctx_sum / H) / max(pc, 1) if pc > 0 else 0.0

    quad = _quad_host(emb.reshape(N, H), lab_f, msk_f)

    loss = ce + 0.5 * float(quad) + 0.1 * ctx
    return np.float32(loss)



# revision 14
# speedup vs baseline: 1.9662x; 1.9662x over previous
"""Trainium2 Bass kernel for nn_NERModel loss (CE + quadruplet + context MSE).

v2 redesign (vs fp32 baseline):
  - All PE matmuls in bf16 (fp32 runs as 2 half-speed passes; bf16 is 4x).
    nat f32 is cast to bf16 once per DMA chunk on VE.
  - Logits computed in NATURAL layout [128 tok, 17] per tile:
    out = embT_c^T @ Wt_c accumulated over 3 K-chunks, + K=1 bias matmul.
    This makes exp/select/ln per-token ops on 128 partitions and kills the
    [17,512] group compaction machinery (selg/woh row-placement matmuls).
  - exp -> expbuf [128, 65*17] bf16; sum/select/ln/weights applied in a few
    BATCHED ops at chunk boundaries / at the end instead of per tile.
  - ctx: dfw (shift-diff) matmul in bf16, Square (no accum) -> sqb bf16,
    per-chunk tensor_reduce -> [128, 8], pair weights applied once at end.
  - Final per-core result is two f32 columns [128, 2] (ce_sum, ctx_sum
    partials); host does the tiny final sums + quadruplet term.

Sharding: data-parallel over batch, 8 batches (8192 tokens) per core.
Tokens tiled 128/tile at stride 127 (65 tiles) so every adjacent-token
pair lands inside some tile; host-built 0/1 weights dedup overlaps.
"""

import os
import sys

for _p in ("/opt/trn_rl_repo", "/root/.axon_site/_ro/trn_rl_repo"):
    if _p not in sys.path:
        sys.path.append(_p)

import numpy as np
import ml_dtypes
from contextlib import ExitStack

import concourse.bass as bass
import concourse.bacc as bacc
import concourse.mybir as mybir
from concourse import tile
from concourse.ap import AP

NUM_LABELS = 17
MARGIN = 1.0
IGNORE = -100

B, S, H, L = 64, 1024, 384, NUM_LABELS
NCORES = 8
BP = B // NCORES            # batches per core
NTOK = BP * S               # tokens per core (8192)
STRIDE = 127                # token stride between tiles (1-token overlap)
NT = 65                     # tiles per core
GDMA = 8                    # tiles per DMA chunk
NDMA = (NT + GDMA - 1) // GDMA  # 9
NPAIR = (NT + 1) // 2       # 33 (last is a singleton)
F32 = mybir.dt.float32
BF16 = mybir.dt.bfloat16

# combined bf16 const tensor layout (columns)
_CO = {}
_off = 0
for _name, _w in [("wtb", 3 * L), ("brow", L), ("idn", 128), ("dfw", 128),
                  ("oneh", NT * L), ("cewT", NT), ("pairwT", NT),
                  ("onesr", 128)]:
    _CO[_name] = (_off, _off + _w)
    _off += _w
CONW = _off


def _tile_start(t: int) -> int:
    # last tile is clamped so it stays in-bounds; duplicated tokens/pairs are
    # zero-weighted on the host side
    return NTOK - 128 if t == NT - 1 else STRIDE * t


def _build_nc() -> bass.Bass:
    nc = bacc.Bacc("TRN2", debug=False)

    emb = nc.declare_dram_parameter("emb", [NTOK, H], F32, isOutput=False)
    conb = nc.declare_dram_parameter("conb", [128, CONW], BF16, isOutput=False)
    outv = nc.declare_dram_parameter("outv", [128, 2], F32, isOutput=True)

    AF = mybir.ActivationFunctionType
    AX = mybir.AxisListType
    OP = mybir.AluOpType
    embt_eng = os.environ.get("NER_EMBT_ENG", "vector")
    # NOTE: DVE cannot read two non-scalar PSUM inputs, so a VE self-multiply
    # of the PSUM diff is illegal — squares run on ScE (activation Square).
    sq_eng = os.environ.get("NER_SQ_ENG", "scalar")
    skip_back = bool(os.environ.get("NER_SKIP_BACK"))
    skip_red = skip_back or bool(os.environ.get("NER_SKIP_RED"))

    with tile.TileContext(nc) as tc, ExitStack() as ctx:
        consts = ctx.enter_context(tc.tile_pool(name="consts", bufs=1))
        nat_pool = ctx.enter_context(tc.tile_pool(name="nat", bufs=3))
        natb_pool = ctx.enter_context(tc.tile_pool(name="natb", bufs=3))
        embtb_pool = ctx.enter_context(tc.tile_pool(name="embtb", bufs=2))
        junk_pool = ctx.enter_context(tc.tile_pool(name="junk", bufs=2))
        acc_pool = ctx.enter_context(tc.tile_pool(name="acc", bufs=1))
        ps_t = ctx.enter_context(tc.tile_pool(name="ps_t", bufs=2, space="PSUM"))
        ps_m = ctx.enter_context(tc.tile_pool(name="ps_m", bufs=2, space="PSUM"))

        con_t = consts.tile([128, CONW], BF16, tag="conb_c")
        nc.sync.dma_start(out=con_t[:], in_=conb.ap())

        def cslice(name, rows=128):
            a, b = _CO[name]
            return con_t[0:rows, a:b]

        # persistent buffers
        sqb = acc_pool.tile([128, NT * H], BF16)       # squared diffs
        expbuf = acc_pool.tile([128, NT * L], BF16)    # exp(logits)
        prodbuf = acc_pool.tile([128, NT * L], BF16)   # exp * onehot
        sumexpb = acc_pool.tile([128, NT], BF16)
        selexpb = acc_pool.tile([128, NT], BF16)
        ctxred = acc_pool.tile([128, NT], BF16)        # per-(slot,tile) ||diff||^2
        lnseb = acc_pool.tile([128, NT], BF16)
        lnselb = acc_pool.tile([128, NT], BF16)
        cedif = acc_pool.tile([128, NT], BF16)
        catbuf = acc_pool.tile([128, 2], F32)

        nat_tiles = {}
        natb_tiles = {}

        def do_dma(d: int):
            ntl = min(GDMA, NT - d * GDMA)
            nat = nat_pool.tile([128, GDMA * H], F32, tag="natbuf")
            if ntl == GDMA:
                src = AP(
                    tensor=emb,
                    offset=_tile_start(d * GDMA) * H,
                    ap=[[H, 128], [STRIDE * H, GDMA], [1, H]],
                )
                nc.sync.dma_start(
                    out=nat[:, :].rearrange("p (g h) -> p g h", h=H), in_=src
                )
            else:
                src = AP(
                    tensor=emb,
                    offset=_tile_start(d * GDMA) * H,
                    ap=[[H, 128], [1, H]],
                )
                nc.sync.dma_start(out=nat[:, 0:H], in_=src)
            nat_tiles[d] = nat

        cast_eng = {
            "gpsimd": nc.gpsimd, "vector": nc.vector
        }[os.environ.get("NER_CAST_ENG", "gpsimd")]

        def do_cast(d: int):
            natb = natb_pool.tile([128, GDMA * H], BF16, tag="natbbuf")
            ntl = min(GDMA, NT - d * GDMA)
            cast_eng.tensor_copy(
                natb[:, 0 : ntl * H], nat_tiles[d][:, 0 : ntl * H]
            )
            natb_tiles[d] = natb

        def natb_slice(t: int, c0: int, c1: int):
            nb = natb_tiles[t // GDMA]
            base = (t % GDMA) * H
            return nb[:, base + c0 : base + c1]

        def pair_tiles(i: int):
            t0 = 2 * i
            return [t0] if t0 == NT - 1 else [t0, t0 + 1]

        embt_ps = {}
        embt_sb = {}
        misc_ps = {}

        def do_front(i: int):
            """transposes + dfw matmuls + embT PSUM->SBUF copy for pair i."""
            tiles = pair_tiles(i)
            ep = ps_t.tile([128, 1024], F32, tag="embt_ps")   # 2 banks
            for j, t in enumerate(tiles):
                for c in range(3):
                    nc.tensor.matmul(
                        ep[:, j * 512 + c * 128 : j * 512 + (c + 1) * 128],
                        natb_slice(t, c * 128, (c + 1) * 128),
                        cslice("idn"),
                        start=True,
                        stop=True,
                    )
            mp = ps_m.tile([128, 1024], F32, tag="misc_ps")   # 2 banks
            for j, t in enumerate(tiles):
                nc.tensor.matmul(
                    mp[:, j * 512 : j * 512 + H],
                    cslice("dfw"),
                    natb_slice(t, 0, H),
                    start=True,
                    stop=True,
                )
            eb = embtb_pool.tile([128, 768], BF16, tag="embt_b")
            nj = len(tiles)
            epv = ep[:, 0 : nj * 512].rearrange("p (j k) -> p j k", k=512)
            ebv = eb[:, 0 : nj * 384].rearrange("p (j k) -> p j k", k=384)
            if embt_eng == "scalar":
                nc.scalar.activation(ebv[:, :, :], epv[:, :, 0:384], AF.Copy)
            else:
                nc.vector.tensor_copy(ebv[:, :, :], epv[:, :, 0:384])
            embt_ps[i] = ep
            embt_sb[i] = eb
            misc_ps[i] = mp

        def do_back(i: int):
            """logits matmuls + exp + squares for pair i."""
            if skip_back:
                return
            tiles = pair_tiles(i)
            eb = embt_sb[i]
            mp = misc_ps[i]
            for j, t in enumerate(tiles):
                lg = mp[:, j * 512 + 384 : j * 512 + 384 + L]
                for c in range(3):
                    nc.tensor.matmul(
                        lg,
                        eb[:, j * 384 + c * 128 : j * 384 + (c + 1) * 128],
                        cslice("wtb")[:, c * L : (c + 1) * L],
                        start=(c == 0),
                        stop=False,
                    )
                nc.tensor.matmul(
                    lg, cslice("onesr", rows=1), cslice("brow", rows=1),
                    start=False, stop=True,
                )
            nj = len(tiles)
            mpv = mp[:, 0 : nj * 512].rearrange("p (j k) -> p j k", k=512)
            ex_out = expbuf[:, i * 2 * L : (i * 2 + nj) * L].rearrange(
                "p (j l) -> p j l", l=L
            )
            nc.scalar.activation(ex_out[:, :, :], mpv[:, :, 384 : 384 + L], AF.Exp)
            sq_out = sqb[:, i * 2 * H : (i * 2 + nj) * H].rearrange(
                "p (j h) -> p j h", h=H
            )
            if sq_eng == "scalar":
                nc.scalar.activation(sq_out[:, :, :], mpv[:, :, 0:H], AF.Square)
            else:
                nc.vector.tensor_tensor(
                    sq_out[:, :, :], mpv[:, :, 0:H], mpv[:, :, 0:H], op=OP.mult
                )

        def do_reduce(d: int):
            """chunk-level reductions for DMA chunk d (tiles 8d..)."""
            if skip_red:
                return
            t0 = d * GDMA
            ntl = min(GDMA, NT - t0)
            with nc.allow_low_precision(reason="bf16 partials within tolerance"):
                nc.vector.tensor_reduce(
                    ctxred[:, t0 : t0 + ntl],
                    sqb[:, t0 * H : (t0 + ntl) * H].rearrange(
                        "p (n h) -> p n h", h=H
                    ),
                    axis=AX.X,
                    op=OP.add,
                )
                nc.vector.tensor_reduce(
                    sumexpb[:, t0 : t0 + ntl],
                    expbuf[:, t0 * L : (t0 + ntl) * L].rearrange(
                        "p (n l) -> p n l", l=L
                    ),
                    axis=AX.X,
                    op=OP.add,
                )
                nc.vector.tensor_tensor(
                    prodbuf[:, t0 * L : (t0 + ntl) * L],
                    expbuf[:, t0 * L : (t0 + ntl) * L],
                    cslice("oneh")[:, t0 * L : (t0 + ntl) * L],
                    op=OP.mult,
                )
                nc.vector.tensor_reduce(
                    selexpb[:, t0 : t0 + ntl],
                    prodbuf[:, t0 * L : (t0 + ntl) * L].rearrange(
                        "p (n l) -> p n l", l=L
                    ),
                    axis=AX.X,
                    op=OP.add,
                )

        # ---- main software-pipelined loop over pairs ----
        do_dma(0)
        do_dma(1)
        do_cast(0)
        for i in range(NPAIR):
            if i % 4 == 0:
                d = i // 4
                if d + 2 < NDMA:
                    do_dma(d + 2)
                if d + 1 < NDMA:
                    do_cast(d + 1)
            do_front(i)
            if i > 0:
                do_back(i - 1)
                if i % 4 == 0:
                    do_reduce(i // 4 - 1)
        do_back(NPAIR - 1)
        do_reduce(NDMA - 1)

        # ---- finals ----
        if skip_red or os.environ.get("NER_SKIP_FIN"):
            nc.vector.memset(catbuf[:], 0.0)
        else:
            nc.scalar.activation(lnseb[:], sumexpb[:], AF.Ln)
            nc.scalar.activation(lnselb[:], selexpb[:], AF.Ln)
            nc.vector.tensor_sub(cedif[:], lnseb[:], lnselb[:])
            junk65 = junk_pool.tile([128, NT], BF16, tag="junk65")
            nc.vector.tensor_mul(junk65[:], cedif[:], cslice("cewT"))
            junk65c = junk_pool.tile([128, NT], BF16, tag="junk65c")
            nc.vector.tensor_scalar(
                out=junk65c[:], in0=junk65[:], scalar1=1.0, scalar2=None,
                op0=OP.mult, op1=OP.add, accum_out=catbuf[:, 0:1],
            )
            junk65b = junk_pool.tile([128, NT], BF16, tag="junk65b")
            nc.vector.tensor_mul(junk65b[:], ctxred[:], cslice("pairwT"))
            junk65d = junk_pool.tile([128, NT], BF16, tag="junk65d")
            nc.vector.tensor_scalar(
                out=junk65d[:], in0=junk65b[:], scalar1=1.0, scalar2=None,
                op0=OP.mult, op1=OP.add, accum_out=catbuf[:, 1:2],
            )
        nc.sync.dma_start(out=outv.ap(), in_=catbuf[:])

    nc.compile()
    return nc


# ---------------------------------------------------------------------------
# host-side preparation


def _host_grids(labf: np.ndarray, mskf: np.ndarray):
    """Per-core grids from labels/mask [NTOK].

    Returns (cewT [128,NT], pairwT [128,NT], oneh [128, NT*L]) as float32;
    caller casts to bf16."""
    valid = labf != IGNORE
    lf = labf.astype(np.int64)
    pair_ok = np.zeros(NTOK, dtype=bool)
    k = np.arange(NTOK - 1)
    in_batch = (k % S) != (S - 1)
    pair_ok[:-1] = in_batch & (lf[:-1] != IGNORE) & (lf[:-1] == lf[1:]) & (lf[:-1] > 0)

    cewT = np.zeros((128, NT), np.float32)
    pairwT = np.zeros((128, NT), np.float32)
    oneh = np.zeros((128, NT * L), np.float32)
    seen_tok = np.zeros(NTOK, dtype=bool)
    seen_pair = np.zeros(NTOK, dtype=bool)
    rows = np.arange(128)
    for t in range(NT):
        s0 = _tile_start(t)
        toks = np.arange(s0, s0 + 128)
        fresh = ~seen_tok[toks]
        cewT[:, t] = (valid[toks] & fresh).astype(np.float32)
        seen_tok[toks] = True
        pfresh = ~seen_pair[toks]
        pw = pair_ok[toks] & pfresh
        pw[127] = False  # col-127 diff is out-of-tile by construction
        pairwT[:, t] = pw.astype(np.float32)
        seen_pair[toks[:127]] = True
        lab_c = np.where(valid[toks], lf[toks], 0)
        oneh[rows, t * L + lab_c] = 1.0
    return cewT, pairwT, oneh


def _quad_host(fe: np.ndarray, fl: np.ndarray, fm: np.ndarray) -> np.float32:
    """Mirror of the reference quadruplet loss in numpy float32."""
    N = fe.shape[0]
    idx = np.arange(N, dtype=np.int64)
    BIG = N
    fm_b = fm > 0
    is_ent = fm_b & (fl > 0)
    non_ent = fm_b & (fl == 0)
    d_i = np.min(np.where(non_ent, idx, BIG))
    has_non = bool(non_ent.any())

    a_i = np.zeros(L - 1, np.int64)
    p_i = np.zeros(L - 1, np.int64)
    n_i = np.zeros(L - 1, np.int64)
    ok = np.zeros(L - 1, bool)
    for i, t in enumerate(range(1, L)):
        m = is_ent & (fl == t)
        order = np.sort(np.where(m, idx, BIG))
        a_i[i], p_i[i] = order[0], order[1]
        cnt = int(m.sum())
        other = is_ent & (fl != t)
        n_i[i] = np.min(np.where(other, idx, BIG))
        ok[i] = (cnt >= 2) and bool(other.any()) and has_non

    clip = lambda v: np.clip(v, 0, N - 1)
    A = fe[clip(a_i)]
    P = fe[clip(p_i)]
    Ng = fe[clip(n_i)]
    D = fe[clip(np.array([d_i]))]
    eps = np.float32(1e-6)

    def dist(x, y):
        d = (x - y + eps).astype(np.float32)
        return np.sqrt(np.sum(d * d, axis=-1, dtype=np.float32)).astype(np.float32)

    pd, nd, dd = dist(A, P), dist(A, Ng), dist(A, D)
    ql = np.maximum(pd - nd + np.float32(MARGIN), 0) + np.maximum(
        pd - dd + np.float32(2.0 * MARGIN), 0
    )
    qcnt = int(ok.sum())
    quad = float(np.sum(np.where(ok, ql, 0.0), dtype=np.float64)) / max(qcnt, 1)
    return np.float32(quad if qcnt > 0 else 0.0)


_NC_CACHE = {}


def _get_nc():
    if "nc" not in _NC_CACHE:
        _NC_CACHE["nc"] = _build_nc()
    return _NC_CACHE["nc"]


def _build_conb(W: np.ndarray, b: np.ndarray, labc: np.ndarray, mskc: np.ndarray):
    """Per-core combined bf16 const tensor [128, CONW]."""
    conb = np.zeros((128, CONW), np.float32)

    def put(name, arr, rows=128):
        a, bb = _CO[name]
        conb[0:rows, a:bb] = arr

    wt = np.zeros((128, 3 * L), np.float32)
    for c in range(3):
        wt[:, c * L : (c + 1) * L] = W[:, c * 128 : (c + 1) * 128].T
    put("wtb", wt)
    put("brow", b.reshape(1, L), rows=1)
    put("idn", np.eye(128, dtype=np.float32))
    dfw = np.zeros((128, 128), np.float32)
    for t in range(127):
        dfw[t + 1, t] = 1.0
    dfw[np.arange(128), np.arange(128)] -= 1.0
    put("dfw", dfw)
    cewT, pairwT, oneh = _host_grids(labc, mskc)
    put("oneh", oneh)
    put("cewT", cewT)
    put("pairwT", pairwT)
    put("onesr", np.ones((1, 128), np.float32), rows=1)
    return conb.astype(ml_dtypes.bfloat16), cewT, pairwT


def kernel(embeddings, classifier_w, classifier_b, labels, attention_mask):
    from concourse.bass_utils import run_bass_kernel_spmd

    emb = np.ascontiguousarray(np.asarray(embeddings, dtype=np.float32))
    W = np.asarray(classifier_w, dtype=np.float32)
    b = np.asarray(classifier_b, dtype=np.float32)
    lab = np.asarray(labels)
    msk = np.asarray(attention_mask)

    lab_f = lab.reshape(-1).astype(np.int64)
    msk_f = msk.reshape(-1).astype(np.int64)
    N = B * S

    in_maps = []
    for cidx in range(NCORES):
        sl = slice(cidx * NTOK, (cidx + 1) * NTOK)
        conb, _, _ = _build_conb(W, b, lab_f[sl], msk_f[sl])
        in_maps.append({"emb": emb.reshape(N, H)[sl], "conb": conb})

    nc = _get_nc()
    res = run_bass_kernel_spmd(nc, in_maps, list(range(NCORES)))

    ce_sum = 0.0
    ctx_sum = 0.0
    for cidx in range(NCORES):
        out = np.asarray(res.results[cidx]["outv"], dtype=np.float64)
        ce_sum += float(out[:, 0].sum())
        ctx_sum += float(out[:, 1].sum())

    valid = lab_f != IGNORE
    ce_cnt = int(valid.sum())
    ce = ce_sum / max(ce_cnt, 1)

    pair_ok = np.zeros(N, dtype=bool)
    k = np.arange(N - 1)
    in_batch = (k % S) != (S - 1)
    pair_ok[:-1] = (
        in_batch & (lab_f[:-1] != IGNORE) & (lab_f[:-1] == lab_f[1:]) & (lab_f[:-1] > 0)
    )
    pc = int(pair_ok.sum())
    ctx = (ctx_sum / H) / max(pc, 1) if pc > 0 else 0.0

    quad = _quad_host(emb.reshape(N, H), lab_f, msk_f)

    loss = ce + 0.5 * float(quad) + 0.1 * ctx
    return np.float32(loss)


# revision 28
# speedup vs baseline: 2.5329x; 1.2882x over previous
"""Trainium2 Bass kernel for nn_NERModel loss (CE + quadruplet + context MSE).

v2 redesign (vs fp32 baseline):
  - All PE matmuls in bf16 (fp32 runs as 2 half-speed passes; bf16 is 4x).
    nat f32 is cast to bf16 once per DMA chunk on VE.
  - Logits computed in NATURAL layout [128 tok, 17] per tile:
    out = embT_c^T @ Wt_c accumulated over 3 K-chunks, + K=1 bias matmul.
    This makes exp/select/ln per-token ops on 128 partitions and kills the
    [17,512] group compaction machinery (selg/woh row-placement matmuls).
  - exp -> expbuf [128, 65*17] bf16; sum/select/ln/weights applied in a few
    BATCHED ops at chunk boundaries / at the end instead of per tile.
  - ctx: dfw (shift-diff) matmul in bf16, Square (no accum) -> sqb bf16,
    per-chunk tensor_reduce -> [128, 8], pair weights applied once at end.
  - Final per-core result is two f32 columns [128, 2] (ce_sum, ctx_sum
    partials); host does the tiny final sums + quadruplet term.

Sharding: data-parallel over batch, 8 batches (8192 tokens) per core.
Tokens tiled 128/tile at stride 127 (65 tiles) so every adjacent-token
pair lands inside some tile; host-built 0/1 weights dedup overlaps.
"""

import os
import sys

for _p in ("/opt/trn_rl_repo", "/root/.axon_site/_ro/trn_rl_repo"):
    if _p not in sys.path:
        sys.path.append(_p)

import numpy as np
import ml_dtypes
from contextlib import ExitStack

import concourse.bass as bass
import concourse.bacc as bacc
import concourse.mybir as mybir
from concourse import tile
from concourse.ap import AP

NUM_LABELS = 17
MARGIN = 1.0
IGNORE = -100

B, S, H, L = 64, 1024, 384, NUM_LABELS
NCORES = 8
BP = B // NCORES            # batches per core
NTOK = BP * S               # tokens per core (8192)
STRIDE = 127                # token stride between tiles (1-token overlap)
NT = 65                     # tiles per core
GDMA = 8                    # tiles per DMA chunk
NDMA = (NT + GDMA - 1) // GDMA  # 9
NPAIR = (NT + 1) // 2       # 33 (last is a singleton)
F32 = mybir.dt.float32
BF16 = mybir.dt.bfloat16

# combined bf16 const tensor layout (columns)
_CO = {}
_off = 0
for _name, _w in [("wtb", 3 * L), ("idn", 128), ("dfw", 128),
                  ("oneh", NT * L), ("expbr", NT * L), ("cewT", NT),
                  ("pairwT", NT)]:
    _CO[_name] = (_off, _off + _w)
    _off += _w
CONW = _off


def _tile_start(t: int) -> int:
    # last tile is clamped so it stays in-bounds; duplicated tokens/pairs are
    # zero-weighted on the host side
    return NTOK - 128 if t == NT - 1 else STRIDE * t


def _build_nc() -> bass.Bass:
    nc = bacc.Bacc("TRN2", debug=False)

    emb = nc.declare_dram_parameter("emb", [NTOK, H], F32, isOutput=False)
    conb = nc.declare_dram_parameter("conb", [128, CONW], BF16, isOutput=False)
    conf = nc.declare_dram_parameter("conf", [128, NT], F32, isOutput=False)
    outv = nc.declare_dram_parameter("outv", [128, 2], F32, isOutput=True)

    AF = mybir.ActivationFunctionType
    AX = mybir.AxisListType
    OP = mybir.AluOpType
    embt_eng = os.environ.get("NER_EMBT_ENG", "vector")
    # NOTE: DVE cannot read two non-scalar PSUM inputs, so a VE self-multiply
    # of the PSUM diff is illegal — squares run on ScE (activation Square).
    sq_eng = os.environ.get("NER_SQ_ENG", "scalar")
    skip_back = bool(os.environ.get("NER_SKIP_BACK"))
    skip_red = skip_back or bool(os.environ.get("NER_SKIP_RED"))

    with tile.TileContext(nc) as tc, ExitStack() as ctx:
        consts = ctx.enter_context(tc.tile_pool(name="consts", bufs=1))
        nat_pool = ctx.enter_context(tc.tile_pool(name="nat", bufs=5))
        natb_pool = ctx.enter_context(tc.tile_pool(name="natb", bufs=3))
        embtb_pool = ctx.enter_context(tc.tile_pool(name="embtb", bufs=2))
        junk_pool = ctx.enter_context(tc.tile_pool(name="junk", bufs=2))
        acc_pool = ctx.enter_context(tc.tile_pool(name="acc", bufs=1))
        ps_t = ctx.enter_context(tc.tile_pool(name="ps_t", bufs=2, space="PSUM"))
        ps_m = ctx.enter_context(tc.tile_pool(name="ps_m", bufs=2, space="PSUM"))

        con_t = consts.tile([128, CONW], BF16, tag="conb_c")
        nc.sync.dma_start(out=con_t[:], in_=conb.ap())
        conf_t = consts.tile([128, NT], F32, tag="conf_c")
        nc.sync.dma_start(out=conf_t[:], in_=conf.ap())

        def cslice(name, rows=128):
            a, b = _CO[name]
            return con_t[0:rows, a:b]

        # persistent buffers
        expbuf = acc_pool.tile([128, NT * L], BF16)    # exp(logits)
        prodbuf = acc_pool.tile([128, NT * L], BF16)   # exp * onehot*exp(b)
        sewbuf = acc_pool.tile([128, NT * L], BF16)    # exp * exp(b)
        sumexpb = acc_pool.tile([128, NT], BF16)
        selexpb = acc_pool.tile([128, NT], BF16)
        ctxcol = acc_pool.tile([128, NT], F32)         # weighted ||diff||^2 sums
        lnseb = acc_pool.tile([128, NT], BF16)
        lnselb = acc_pool.tile([128, NT], BF16)
        cedif = acc_pool.tile([128, NT], BF16)
        catbuf = acc_pool.tile([128, 2], F32)

        nat_tiles = {}
        natb_tiles = {}

        def do_dma(d: int):
            ntl = min(GDMA, NT - d * GDMA)
            nat = nat_pool.tile([128, GDMA * H], F32, tag="natbuf")
            if ntl == GDMA:
                src = AP(
                    tensor=emb,
                    offset=_tile_start(d * GDMA) * H,
                    ap=[[H, 128], [STRIDE * H, GDMA], [1, H]],
                )
                nc.sync.dma_start(
                    out=nat[:, :].rearrange("p (g h) -> p g h", h=H), in_=src
                )
            else:
                src = AP(
                    tensor=emb,
                    offset=_tile_start(d * GDMA) * H,
                    ap=[[H, 128], [1, H]],
                )
                nc.sync.dma_start(out=nat[:, 0:H], in_=src)
            nat_tiles[d] = nat

        cast_eng = {
            "gpsimd": nc.gpsimd, "vector": nc.vector
        }[os.environ.get("NER_CAST_ENG", "vector")]

        def do_cast(d: int):
            natb = natb_pool.tile([128, GDMA * H], BF16, tag="natbbuf")
            ntl = min(GDMA, NT - d * GDMA)
            cast_eng.tensor_copy(
                natb[:, 0 : ntl * H], nat_tiles[d][:, 0 : ntl * H]
            )
            natb_tiles[d] = natb

        def natb_slice(t: int, c0: int, c1: int):
            nb = natb_tiles[t // GDMA]
            base = (t % GDMA) * H
            return nb[:, base + c0 : base + c1]

        def pair_tiles(i: int):
            t0 = 2 * i
            return [t0] if t0 == NT - 1 else [t0, t0 + 1]

        embt_ps = {}
        embt_sb = {}
        misc_ps = {}

        def do_front(i: int):
            """transposes + dfw matmuls + embT PSUM->SBUF copy for pair i."""
            tiles = pair_tiles(i)
            ep = ps_t.tile([128, 1024], F32, tag="embt_ps")   # 2 banks
            for j, t in enumerate(tiles):
                for c in range(3):
                    nc.tensor.matmul(
                        ep[:, j * 512 + c * 128 : j * 512 + (c + 1) * 128],
                        natb_slice(t, c * 128, (c + 1) * 128),
                        cslice("idn"),
                        start=True,
                        stop=True,
                    )
            mp = ps_m.tile([128, 1024], F32, tag="misc_ps")   # 2 banks
            for j, t in enumerate(tiles):
                nc.tensor.matmul(
                    mp[:, j * 512 : j * 512 + H],
                    cslice("dfw"),
                    natb_slice(t, 0, H),
                    start=True,
                    stop=True,
                )
            eb = embtb_pool.tile([128, 768], BF16, tag="embt_b")
            nj = len(tiles)
            epv = ep[:, 0 : nj * 512].rearrange("p (j k) -> p j k", k=512)
            ebv = eb[:, 0 : nj * 384].rearrange("p (j k) -> p j k", k=384)
            if embt_eng == "scalar":
                nc.scalar.activation(ebv[:, :, :], epv[:, :, 0:384], AF.Copy)
            else:
                nc.vector.tensor_copy(ebv[:, :, :], epv[:, :, 0:384])
            embt_ps[i] = ep
            embt_sb[i] = eb
            misc_ps[i] = mp

        def do_back(i: int):
            """logits matmuls + exp + squares for pair i."""
            if skip_back:
                return
            tiles = pair_tiles(i)
            eb = embt_sb[i]
            mp = misc_ps[i]
            for j, t in enumerate(tiles):
                lg = mp[:, j * 512 + 384 : j * 512 + 384 + L]
                for c in range(3):
                    nc.tensor.matmul(
                        lg,
                        eb[:, j * 384 + c * 128 : j * 384 + (c + 1) * 128],
                        cslice("wtb")[:, c * L : (c + 1) * L],
                        start=(c == 0),
                        stop=(c == 2),
                    )
            nj = len(tiles)
            mpv = mp[:, 0 : nj * 512].rearrange("p (j k) -> p j k", k=512)
            ex_out = expbuf[:, i * 2 * L : (i * 2 + nj) * L].rearrange(
                "p (j l) -> p j l", l=L
            )
            nc.scalar.activation(ex_out[:, :, :], mpv[:, :, 384 : 384 + L], AF.Exp)
            # per-tile weighted squares: (pairw * diff)^2 accumulated over H
            # into ctxcol — pairw is 0/1 so scale^2 == scale
            for j, t in enumerate(tiles):
                jk = junk_pool.tile([128, H], BF16, tag="junkS")
                nc.scalar.activation(
                    jk[:],
                    mp[:, j * 512 : j * 512 + H],
                    AF.Square,
                    bias=0.0,
                    scale=conf_t[:, t : t + 1],
                    accum_out=ctxcol[:, t : t + 1],
                )

        def do_reduce(d: int):
            """chunk-level reductions for DMA chunk d (tiles 8d..)."""
            if skip_red:
                return
            t0 = d * GDMA
            ntl = min(GDMA, NT - t0)
            sl = slice(t0 * L, (t0 + ntl) * L)
            # exp(b) weighting for sumexp and sel (b==0 -> multiply by 1)
            nc.gpsimd.tensor_tensor(
                sewbuf[:, sl], expbuf[:, sl], cslice("expbr")[:, sl], op=OP.mult
            )
            nc.gpsimd.tensor_tensor(
                prodbuf[:, sl], expbuf[:, sl], cslice("oneh")[:, sl], op=OP.mult
            )
            with nc.allow_low_precision(reason="bf16 partials within tolerance"):
                nc.vector.tensor_reduce(
                    sumexpb[:, t0 : t0 + ntl],
                    sewbuf[:, sl].rearrange("p (n l) -> p n l", l=L),
                    axis=AX.X,
                    op=OP.add,
                )
                nc.vector.tensor_reduce(
                    selexpb[:, t0 : t0 + ntl],
                    prodbuf[:, sl].rearrange("p (n l) -> p n l", l=L),
                    axis=AX.X,
                    op=OP.add,
                )

        # ---- main software-pipelined loop over pairs ----
        do_dma(0)
        do_dma(1)
        do_dma(2)
        do_cast(0)
        for i in range(NPAIR):
            if i % 4 == 0:
                d = i // 4
                if d + 3 < NDMA:
                    do_dma(d + 3)
                if d + 1 < NDMA:
                    do_cast(d + 1)
            do_front(i)
            if i > 0:
                do_back(i - 1)
                if i % 4 == 0:
                    do_reduce(i // 4 - 1)
        do_back(NPAIR - 1)
        do_reduce(NDMA - 1)

        # ---- finals ----
        if skip_red or os.environ.get("NER_SKIP_FIN"):
            nc.vector.memset(catbuf[:], 0.0)
        else:
            nc.scalar.activation(lnseb[:], sumexpb[:], AF.Ln)
            nc.scalar.activation(lnselb[:], selexpb[:], AF.Ln)
            nc.vector.tensor_sub(cedif[:], lnseb[:], lnselb[:])
            junk65 = junk_pool.tile([128, NT], BF16, tag="junk65")
            nc.vector.tensor_mul(junk65[:], cedif[:], cslice("cewT"))
            junk65c = junk_pool.tile([128, NT], BF16, tag="junk65c")
            nc.vector.tensor_scalar(
                out=junk65c[:], in0=junk65[:], scalar1=1.0, scalar2=None,
                op0=OP.mult, op1=OP.add, accum_out=catbuf[:, 0:1],
            )
            junk65d = junk_pool.tile([128, NT], F32, tag="junk65d")
            nc.vector.tensor_scalar(
                out=junk65d[:], in0=ctxcol[:], scalar1=1.0, scalar2=None,
                op0=OP.mult, op1=OP.add, accum_out=catbuf[:, 1:2],
            )
        nc.sync.dma_start(out=outv.ap(), in_=catbuf[:])

    nc.compile()
    return nc


# ---------------------------------------------------------------------------
# host-side preparation


def _host_grids(labf: np.ndarray, mskf: np.ndarray, b: np.ndarray):
    """Per-core grids from labels/mask [NTOK].

    Returns (cewT [128,NT], pairwT [128,NT], oneh [128, NT*L]) as float32;
    caller casts to bf16. oneh carries exp(b[label]) at the label slot so
    ln(sel) == logit + bias with no device-side bias add."""
    valid = labf != IGNORE
    lf = labf.astype(np.int64)
    expb = np.exp(b.astype(np.float64)).astype(np.float32)
    pair_ok = np.zeros(NTOK, dtype=bool)
    k = np.arange(NTOK - 1)
    in_batch = (k % S) != (S - 1)
    pair_ok[:-1] = in_batch & (lf[:-1] != IGNORE) & (lf[:-1] == lf[1:]) & (lf[:-1] > 0)

    cewT = np.zeros((128, NT), np.float32)
    pairwT = np.zeros((128, NT), np.float32)
    oneh = np.zeros((128, NT * L), np.float32)
    seen_tok = np.zeros(NTOK, dtype=bool)
    seen_pair = np.zeros(NTOK, dtype=bool)
    rows = np.arange(128)
    for t in range(NT):
        s0 = _tile_start(t)
        toks = np.arange(s0, s0 + 128)
        fresh = ~seen_tok[toks]
        cewT[:, t] = (valid[toks] & fresh).astype(np.float32)
        seen_tok[toks] = True
        pfresh = ~seen_pair[toks]
        pw = pair_ok[toks] & pfresh
        pw[127] = False  # col-127 diff is out-of-tile by construction
        pairwT[:, t] = pw.astype(np.float32)
        seen_pair[toks[:127]] = True
        lab_c = np.where(valid[toks], lf[toks], 0)
        oneh[rows, t * L + lab_c] = expb[lab_c]
    return cewT, pairwT, oneh


def _quad_host(fe: np.ndarray, fl: np.ndarray, fm: np.ndarray) -> np.float32:
    """Mirror of the reference quadruplet loss in numpy float32."""
    N = fe.shape[0]
    idx = np.arange(N, dtype=np.int64)
    BIG = N
    fm_b = fm > 0
    is_ent = fm_b & (fl > 0)
    non_ent = fm_b & (fl == 0)
    d_i = np.min(np.where(non_ent, idx, BIG))
    has_non = bool(non_ent.any())

    a_i = np.zeros(L - 1, np.int64)
    p_i = np.zeros(L - 1, np.int64)
    n_i = np.zeros(L - 1, np.int64)
    ok = np.zeros(L - 1, bool)
    for i, t in enumerate(range(1, L)):
        m = is_ent & (fl == t)
        order = np.sort(np.where(m, idx, BIG))
        a_i[i], p_i[i] = order[0], order[1]
        cnt = int(m.sum())
        other = is_ent & (fl != t)
        n_i[i] = np.min(np.where(other, idx, BIG))
        ok[i] = (cnt >= 2) and bool(other.any()) and has_non

    clip = lambda v: np.clip(v, 0, N - 1)
    A = fe[clip(a_i)]
    P = fe[clip(p_i)]
    Ng = fe[clip(n_i)]
    D = fe[clip(np.array([d_i]))]
    eps = np.float32(1e-6)

    def dist(x, y):
        d = (x - y + eps).astype(np.float32)
        return np.sqrt(np.sum(d * d, axis=-1, dtype=np.float32)).astype(np.float32)

    pd, nd, dd = dist(A, P), dist(A, Ng), dist(A, D)
    ql = np.maximum(pd - nd + np.float32(MARGIN), 0) + np.maximum(
        pd - dd + np.float32(2.0 * MARGIN), 0
    )
    qcnt = int(ok.sum())
    quad = float(np.sum(np.where(ok, ql, 0.0), dtype=np.float64)) / max(qcnt, 1)
    return np.float32(quad if qcnt > 0 else 0.0)


_NC_CACHE = {}


def _get_nc():
    if "nc" not in _NC_CACHE:
        _NC_CACHE["nc"] = _build_nc()
    return _NC_CACHE["nc"]


def _build_conb(W: np.ndarray, b: np.ndarray, labc: np.ndarray, mskc: np.ndarray):
    """Per-core combined bf16 const tensor [128, CONW]."""
    conb = np.zeros((128, CONW), np.float32)

    def put(name, arr, rows=128):
        a, bb = _CO[name]
        conb[0:rows, a:bb] = arr

    wt = np.zeros((128, 3 * L), np.float32)
    for c in range(3):
        wt[:, c * L : (c + 1) * L] = W[:, c * 128 : (c + 1) * 128].T
    put("wtb", wt)
    put("idn", np.eye(128, dtype=np.float32))
    dfw = np.zeros((128, 128), np.float32)
    for t in range(127):
        dfw[t + 1, t] = 1.0
    dfw[np.arange(128), np.arange(128)] -= 1.0
    put("dfw", dfw)
    cewT, pairwT, oneh = _host_grids(labc, mskc, b)
    put("oneh", oneh)
    expb = np.exp(b.astype(np.float64)).astype(np.float32)
    put("expbr", np.tile(expb, NT).reshape(1, NT * L).repeat(128, axis=0))
    put("cewT", cewT)
    put("pairwT", pairwT)
    return conb.astype(ml_dtypes.bfloat16), cewT, pairwT


def kernel(embeddings, classifier_w, classifier_b, labels, attention_mask):
    from concourse.bass_utils import run_bass_kernel_spmd

    emb = np.ascontiguousarray(np.asarray(embeddings, dtype=np.float32))
    W = np.asarray(classifier_w, dtype=np.float32)
    b = np.asarray(classifier_b, dtype=np.float32)
    lab = np.asarray(labels)
    msk = np.asarray(attention_mask)

    lab_f = lab.reshape(-1).astype(np.int64)
    msk_f = msk.reshape(-1).astype(np.int64)
    N = B * S

    in_maps = []
    for cidx in range(NCORES):
        sl = slice(cidx * NTOK, (cidx + 1) * NTOK)
        conb, _, pairwT = _build_conb(W, b, lab_f[sl], msk_f[sl])
        in_maps.append({"emb": emb.reshape(N, H)[sl], "conb": conb,
                        "conf": np.ascontiguousarray(pairwT)})

    nc = _get_nc()
    res = run_bass_kernel_spmd(nc, in_maps, list(range(NCORES)))

    ce_sum = 0.0
    ctx_sum = 0.0
    for cidx in range(NCORES):
        out = np.asarray(res.results[cidx]["outv"], dtype=np.float64)
        ce_sum += float(out[:, 0].sum())
        ctx_sum += float(out[:, 1].sum())

    valid = lab_f != IGNORE
    ce_cnt = int(valid.sum())
    ce = ce_sum / max(ce_cnt, 1)

    pair_ok = np.zeros(N, dtype=bool)
    k = np.arange(N - 1)
    in_batch = (k % S) != (S - 1)
    pair_ok[:-1] = (
        in_batch & (lab_f[:-1] != IGNORE) & (lab_f[:-1] == lab_f[1:]) & (lab_f[:-1] > 0)
    )
    pc = int(pair_ok.sum())
    ctx = (ctx_sum / H) / max(pc, 1) if pc > 0 else 0.0

    quad = _quad_host(emb.reshape(N, H), lab_f, msk_f)

    loss = ce + 0.5 * float(quad) + 0.1 * ctx
    return np.float32(loss)
